# revision 50
# baseline (speedup 1.0000x reference)
"""Trainium2 Bass kernel for the CIN-style layer:

    z   = einsum('btf,byf->bfty', x_0, x_k)            # pairwise outer products
    z   = z.reshape(bs, ts0, f, tsk)                   # flat reinterpretation
    out = einsum('btiy,nty->bni', z, conv_w) + conv_b  # strided conv reduction

Shapes: x_0 (32, 64, 256), x_k (32, 64, 256), conv_w (128, 64, 64),
conv_b (128,) -> out (32, 128, 256).

Math: with i = a*64 + m  (a = i//64, m = i%64) and feature f = 4t + a the
reference reduces to a two-step factorization (verified to 7e-7 rel err):

    W2[b,n,t,a]      = sum_y x_k[b,y,4t+a] * conv_w[n,t,y]         (contract y)
    out[b,n,a*64+m]  = sum_t x_0[b,m,4t+a] * W2[b,n,t,a] + conv_b  (contract t)

This is ~270 MFLOP total vs 8.6 GFLOP for the naive path.

Sharding: pure data parallel over batch, 4 samples per core, conv_w/conv_b
replicated (no collectives).

v11 (bf16 everywhere, split bounce, dual HWDGE rings):
  All device data is bf16 (PSUM accumulation stays fp32); the rel-err
  gate is 2e-2 and bf16 lands ~2e-3.  xk/x0 ship dense and are zero-padded
  into block-diagonal lhsT layouts on-chip (DVE), halving input bytes.

  step 1 (contract y): per t-pair p = (k, j), stationary lhsT = padded xk
    tile [K=128 (q,y), M=32 (q',c)], moving rhs = conv_w tile [128, 128 n];
    4 pairs (j) per pass k via column tiling -> PSUM [32j+16q'+2c2+h, n].
  shuffle: step 2 needs t-partitioned operands; W2 takes a DRAM bounce.
    Bounce layout [j,q,c2,h,k,n] makes the WRITE side contiguous per
    partition (the k-half of a row = 1KB run) and puts the 256B-run
    gather on the read side (no sub-512B write RMW).  Step-2's
    contraction index is ordered kappa = (k2, h, k1k0, j, q) so each
    k-half of the bounce lands in a contiguous 64-partition block:
    the k<4 half readback (and the first K=64 accumulation pass of
    step 2) overlaps the k>=4 half of step 1 + its bounce write.
  step 2 (contract t): per c2 = (b, a1): two accumulating matmuls
    [K=64 (kappa half), M=128 (h',m)] x [K=64, 128 n] -> PSUM [64h'+m, n];
    bias fused into the PSUM->SBUF copy (DVE), output stored bf16 and
    upcast host-side.

All layout work (transposes, dense packing, final unshuffle) happens
host-side in numpy; the device only runs contiguous-ish DMAs, matmuls,
memsets and DVE copies.
"""

import numpy as np
import ml_dtypes

BS, TS, F, NF = 32, 64, 256, 128
NCORES = 8
B = BS // NCORES  # 4 local batches per core

F32 = np.float32
BF16 = ml_dtypes.bfloat16


# ---------------------------------------------------------------------------
# Host-side packing
# ---------------------------------------------------------------------------

def _pack_wt(conv_w: np.ndarray) -> np.ndarray:
    # WT[64q+y, 128p+n] = conv_w[n, 2p+q, y]
    wt = conv_w.transpose(1, 2, 0).reshape(32, 2, 64, NF)  # [p, q, y, n]
    wt = wt.transpose(1, 2, 0, 3)                          # [q, y, p, n]
    return np.ascontiguousarray(wt.reshape(128, 32 * NF), dtype=BF16)


def _pack_xk(xk_shard: np.ndarray) -> np.ndarray:
    # dense: XKD[64q+y, 16p+c] = xk[b, y, 8p+4q+a]   (c = 4b+a)
    xq = xk_shard.reshape(B, TS, 32, 2, 4)       # [b, y, p, q, a]
    src = xq.transpose(3, 1, 2, 0, 4)            # [q, y, p, b, a]
    return np.ascontiguousarray(src.reshape(128, 512), dtype=BF16)


def _pack_x0(x0_shard: np.ndarray) -> np.ndarray:
    # padded: X0L[kappa, (c2,h',m)] = x0[b, m, 4t+a] iff h'==h(kappa),
    # with c = 2c2+h = 4b+a, kappa = 64k2+32h+8k1k0+2j+q, t = 32k2+8k1k0+2j+q
    xt = x0_shard.reshape(B, TS, TS, 4).transpose(0, 3, 2, 1)  # [b, a, t, m]
    flat = xt.reshape(16, TS, TS)                              # [c, t, m]
    f = flat.reshape(8, 2, 2, 4, 4, 2, TS)       # [c2, h, k2, k10, j, q, m]
    x0d = f.transpose(2, 1, 3, 4, 5, 0, 6)       # [k2, h, k10, j, q, c2, m]
    x0l = np.zeros((2, 2, 32, 8, 2, TS), dtype=BF16)  # [k2,h,rest,c2,h',m]
    for h in range(2):
        x0l[:, h, :, :, h, :] = x0d.reshape(2, 2, 32, 8, TS)[:, h]
    return np.ascontiguousarray(x0l.reshape(128, 1024))


def _unpack_out(out_pack: np.ndarray, out_full: np.ndarray, r: int) -> None:
    # out_pack[64h+m, 128c2+n] = out[4r+b(c), n, a(c)*64+m], c = 2*c2+h
    o = np.asarray(out_pack, dtype=F32).reshape(2, TS, 8, NF)  # [h, m, c2, n]
    for c2 in range(8):
        for h in (0, 1):
            c = 2 * c2 + h
            b, a = divmod(c, 4)
            out_full[4 * r + b, :, a * TS:(a + 1) * TS] = o[h, :, c2, :].T


# ---------------------------------------------------------------------------
# Device program
# ---------------------------------------------------------------------------

_prog_cache = {}


def _emit_body_v11(nc, tc, pool, ps_pool, ps_pool2, in0_d, in1_d, out_d,
                   w2b_d):
    import concourse.mybir as mybir

    bf16 = mybir.dt.bfloat16
    f32 = mybir.dt.float32

    # PE warm-up: back-to-back matmuls on a zeroed bf16 tile while the
    # input DMAs stream in; gets the HAM clock gate to 2.4GHz before
    # step 1 starts.  No data deps -> scheduled first on PE.
    warm_s = pool.tile([128, 512], bf16, tag="warm")
    nc.gpsimd.memset(warm_s[:], 0.0)
    ps_w = ps_pool.tile([128, 512], f32, tag="warm_ps")
    for _ in range(6):
        nc.tensor.matmul(ps_w[:, :], warm_s[:, 0:128], warm_s[:, :],
                         start=True, stop=True)

    # ---- input DMAs split across both HWDGE rings so the two streams
    # run concurrently; chunks sized so both rings finish ~together ----
    wta = pool.tile([128, 1536], bf16, tag="wta")     # xk dense | wt pairs 0-7
    nc.sync.dma_start(wta[:], in0_d.ap()[:, 0:1536])
    wtb = pool.tile([128, 2048], bf16, tag="wtb")     # wt pairs 8-23
    nc.scalar.dma_start(wtb[:], in0_d.ap()[:, 1536:3584])
    wtc = pool.tile([128, 1024], bf16, tag="wtc")     # wt pairs 24-31
    nc.sync.dma_start(wtc[:], in0_d.ap()[:, 3584:4608])
    in1_s = pool.tile([128, 1152], bf16, tag="in1")   # x0 padded | bias
    nc.scalar.dma_start(in1_s[:], in1_d.ap())
    x0_pad = in1_s[:, 0:1024]

    # ---- zero-pad dense xk into its block-diagonal lhsT layout (DVE,
    # gates step-1; x0 ships pre-padded so step-2's stationary needs no
    # on-chip work that could contend with the psum casts) ----
    xk_pad = pool.tile([128, 1024], bf16, tag="xkpad")
    nc.gpsimd.memset(xk_pad[:], 0.0)
    for q in range(2):
        dst = xk_pad[64 * q:64 * (q + 1), :].rearrange(
            "p (a b) -> p a b", b=32)[:, :, 16 * q:16 * (q + 1)]
        src = wta[64 * q:64 * (q + 1), 0:512].rearrange(
            "p (a b) -> p a b", b=16)
        nc.vector.tensor_copy(dst, src)
    bias_f = pool.tile([128, 128], f32, tag="bias")

    def wt_cols(p):  # rhs tile [128, 128] for pair p
        if p < 8:
            return wta[:, 512 + 128 * p:512 + 128 * (p + 1)]
        if p < 24:
            return wtb[:, 128 * (p - 8):128 * (p - 7)]
        return wtc[:, 128 * (p - 24):128 * (p - 23)]

    # ---- step 1: W2 = xk . wT, contract y (K = 128 = (q, y)) ----
    # psum tile u holds passes 4u..4u+3 at col 128*(k%4); each half is
    # cast-copied to bf16 and bounced out (contiguous 1KB runs) while
    # the other half computes.
    w2_s = pool.tile([128, 1024], bf16, tag="w2")
    w2r_s = pool.tile([128, 1024], bf16, tag="w2r")

    for u in range(2):
        ps1 = ps_pool.tile([128, 512], f32, tag="s1")
        for k in range(4 * u, 4 * u + 4):
            for j in range(4):
                p = 4 * k + j
                nc.tensor.matmul(
                    ps1[32 * j:32 * (j + 1), 128 * (k % 4):128 * (k % 4 + 1)],
                    xk_pad[:, 32 * p:32 * (p + 1)],
                    wt_cols(p),
                    start=True,
                    stop=True,
                    tile_position=(0, 32 * j),
                )
        nc.vector.tensor_copy(w2_s[:, 512 * u:512 * (u + 1)], ps1[:, :])
        # bounce out half u in two j-half writes on both rings at once:
        # src [p=(j,q,c2,h), (k,n)] scatters into the kappa-major dram
        # layout [h,k,j,q,c2,n] (k2 = u slice)
        for v in range(2):
            srcA = w2_s[64 * v:64 * (v + 1),
                        512 * u:512 * (u + 1)].rearrange(
                "p (k n) -> p k n", k=4)
            dstA = w2b_d.ap()[u, :, :, 2 * v:2 * (v + 1)].rearrange(
                "h k j q c2 n -> j q c2 h k n")
            (nc.sync if v == 0 else nc.scalar).dma_start(dstA, srcA)
    for u in range(2):
        # readback half u: kappa-major layout makes this a plain
        # contiguous [64, 1024] load into partitions [64u, 64u+64);
        # the two halves stream on both rings concurrently
        (nc.sync if u == 0 else nc.scalar).dma_start(
            w2r_s[64 * u:64 * (u + 1), :], w2b_d.ap()[u])

    # bias cast: small DVE op for the bounce-latency window
    nc.vector.tensor_copy(bias_f[:], in1_s[:, 1024:1152])

    # ---- step 2: out = x0 . W2, contract t (K = 128 = kappa) ----
    # quarter-bank psum tiles so each bias-add + out DMA fires after
    # just two matmuls
    out_s = pool.tile([128, 1024], bf16, tag="out")
    bias2 = bias_f.unsqueeze(1).broadcast_to([128, 2, 128])
    for v in range(4):
        ps2 = ps_pool2.tile([128, 256], f32, tag="s2")
        for c2 in range(2 * v, 2 * v + 2):
            nc.tensor.matmul(
                ps2[:, 128 * (c2 % 2):128 * (c2 % 2 + 1)],
                x0_pad[:, 128 * c2:128 * (c2 + 1)],
                w2r_s[:, 128 * c2:128 * (c2 + 1)],
                start=True,
                stop=True,
            )
        nc.vector.tensor_add(
            out_s[:, 256 * v:256 * (v + 1)].rearrange("p (f n) -> p f n", f=2),
            ps2[:, :].rearrange("p (f n) -> p f n", f=2),
            bias2,
        )
        sl = slice(256 * v, 256 * (v + 1))
        (nc.sync if v % 2 == 0 else nc.scalar).dma_start(
            out_d.ap()[:, sl], out_s[:, sl])


def _build_program(version=20):
    if version in _prog_cache:
        return _prog_cache[version]

    from contextlib import ExitStack

    import concourse.bacc as bacc
    import concourse.mybir as mybir
    import concourse.tile as tile

    bf16 = mybir.dt.bfloat16
    nc = bacc.Bacc("TRN2", target_bir_lowering=False, debug=False)

    # in0 = [xk_dense (512) | wt (4096)], in1 = [x0_dense (512) | bias (128)]
    in0_d = nc.dram_tensor("in0_pack", [128, 4608], bf16, kind="ExternalInput")
    in1_d = nc.dram_tensor("in1_pack", [128, 1152], bf16, kind="ExternalInput")
    out_d = nc.dram_tensor("out_pack", [128, 1024], bf16, kind="ExternalOutput")
    # bounce layout [k2, h, k10, j, q, c2, n] (kappa-major)
    w2b_d = nc.dram_tensor("w2_bounce", [2, 2, 4, 4, 2, 8, 128], bf16)

    with tile.TileContext(nc) as tc, ExitStack() as ctx:
        pool = ctx.enter_context(tc.tile_pool(name="io", bufs=1))
        ps_pool = ctx.enter_context(tc.tile_pool(name="ps", bufs=2, space="PSUM"))
        ps_pool2 = ctx.enter_context(tc.tile_pool(name="ps2", bufs=4, space="PSUM"))
        _emit_body_v11(nc, tc, pool, ps_pool, ps_pool2, in0_d, in1_d, out_d,
                       w2b_d)

    nc.compile()
    _prog_cache[version] = nc
    return nc


def pack_core_inputs(x_0, x_k, conv_w, conv_b, version=20):
    """Returns (in_maps list of 8 dicts) for run_bass_kernel_spmd."""
    wt = _pack_wt(np.asarray(conv_w, dtype=F32))
    bias = np.ascontiguousarray(
        np.broadcast_to(
            np.asarray(conv_b, dtype=F32).astype(BF16), (128, 128)))
    x0 = np.asarray(x_0, dtype=F32)
    xk = np.asarray(x_k, dtype=F32)
    in_maps = []
    for r in range(NCORES):
        in0 = np.concatenate([_pack_xk(xk[B * r:B * (r + 1)]), wt], axis=1)
        in1 = np.concatenate([_pack_x0(x0[B * r:B * (r + 1)]), bias], axis=1)
        in_maps.append({
            "in0_pack": np.ascontiguousarray(in0),
            "in1_pack": np.ascontiguousarray(in1),
        })
    return in_maps


VERSION = 20  # current best variant (= v17/v15 structure)


def kernel(x_0, x_k, conv_w, conv_b):
    from concourse.bass_utils import run_bass_kernel_spmd

    nc = _build_program(VERSION)
    in_maps = pack_core_inputs(x_0, x_k, conv_w, conv_b, version=VERSION)
    res = run_bass_kernel_spmd(nc, in_maps, core_ids=list(range(NCORES)))
    out = np.empty((BS, NF, F), dtype=F32)
    for r in range(NCORES):
        _unpack_out(res.results[r]["out_pack"], out, r)
    return out


# ---------------------------------------------------------------------------
# numpy model of the packed device program (for testing the packing logic)
# ---------------------------------------------------------------------------

def _numpy_model(x_0, x_k, conv_w, conv_b):
    out = np.empty((BS, NF, F), dtype=F32)
    in_maps = pack_core_inputs(x_0, x_k, conv_w, conv_b)
    for r in range(NCORES):
        m = in_maps[r]
        xkd = np.asarray(m["in0_pack"][:, :512], dtype=F32)
        wt = np.asarray(m["in0_pack"][:, 512:], dtype=F32)
        x0_pad = np.asarray(m["in1_pack"][:, :1024], dtype=F32)
        bias = np.asarray(m["in1_pack"][:, 1024:1152], dtype=F32)
        # on-chip xk pad
        xk_pad = np.zeros((128, 1024), dtype=F32)
        for q in range(2):
            s = xkd[64 * q:64 * (q + 1)].reshape(64, 32, 16)
            d = xk_pad[64 * q:64 * (q + 1)].reshape(64, 32, 32)
            d[:, :, 16 * q:16 * (q + 1)] = s
        # step 1
        w2 = np.zeros((128, 1024), dtype=F32)
        for k in range(8):
            for j in range(4):
                p = 4 * k + j
                w2[32 * j:32 * (j + 1), 128 * k:128 * (k + 1)] = (
                    xk_pad[:, 32 * p:32 * (p + 1)].T
                    @ wt[:, 128 * p:128 * (p + 1)])
        # bounce: src partition (j,q,c2,h), free (k,n) -> [j,q,c2,h,k,n]
        w2b = w2.reshape(4, 2, 8, 2, 8, 128)
        # readback: dst partition (h,k,j,q) with k = 4*k2 + k10 and the
        # k2 bit on top: kappa = (k2, h, k10, j, q)
        w2r = np.empty((2, 2, 4, 4, 2, 8, 128), dtype=F32)  # [k2,h,k10,j,q,c2,n]
        for k2 in range(2):
            # srcB slice [j,q,c2,h,k10,n] -> (h k10 j q) c2 n
            sl = w2b[:, :, :, :, 4 * k2:4 * (k2 + 1), :]
            w2r[k2] = sl.transpose(3, 4, 0, 1, 2, 5)
        w2r = w2r.reshape(128, 1024)
        # step 2
        out_pack = np.zeros((128, 1024), dtype=F32)
        for c2 in range(8):
            out_pack[:, 128 * c2:128 * (c2 + 1)] = (
                x0_pad[:, 128 * c2:128 * (c2 + 1)].T
                @ w2r[:, 128 * c2:128 * (c2 + 1)] + bias)
        _unpack_out(out_pack, out, r)
    return out


# revision 51
# speedup vs baseline: 1.1432x; 1.1432x over previous
"""Trainium2 Bass kernel for the CIN-style layer:

    z   = einsum('btf,byf->bfty', x_0, x_k)            # pairwise outer products
    z   = z.reshape(bs, ts0, f, tsk)                   # flat reinterpretation
    out = einsum('btiy,nty->bni', z, conv_w) + conv_b  # strided conv reduction

Shapes: x_0 (32, 64, 256), x_k (32, 64, 256), conv_w (128, 64, 64),
conv_b (128,) -> out (32, 128, 256).

Math: with i = a*64 + m  (a = i//64, m = i%64) and feature f = 4t + a the
reference reduces to a two-step factorization (verified to 7e-7 rel err):

    W2[b,n,t,a]      = sum_y x_k[b,y,4t+a] * conv_w[n,t,y]         (contract y)
    out[b,n,a*64+m]  = sum_t x_0[b,m,4t+a] * W2[b,n,t,a] + conv_b  (contract t)

This is ~270 MFLOP total vs 8.6 GFLOP for the naive path.

Sharding: pure data parallel over batch, 4 samples per core, conv_w/conv_b
replicated (no collectives).

v11 (bf16 everywhere, split bounce, dual HWDGE rings):
  All device data is bf16 (PSUM accumulation stays fp32); the rel-err
  gate is 2e-2 and bf16 lands ~2e-3.  xk/x0 ship dense and are zero-padded
  into block-diagonal lhsT layouts on-chip (DVE), halving input bytes.

  step 1 (contract y): per t-pair p = (k, j), stationary lhsT = padded xk
    tile [K=128 (q,y), M=32 (q',c)], moving rhs = conv_w tile [128, 128 n];
    4 pairs (j) per pass k via column tiling -> PSUM [32j+16q'+2c2+h, n].
  shuffle: step 2 needs t-partitioned operands; W2 takes a DRAM bounce.
    Bounce layout [j,q,c2,h,k,n] makes the WRITE side contiguous per
    partition (the k-half of a row = 1KB run) and puts the 256B-run
    gather on the read side (no sub-512B write RMW).  Step-2's
    contraction index is ordered kappa = (k2, h, k1k0, j, q) so each
    k-half of the bounce lands in a contiguous 64-partition block:
    the k<4 half readback (and the first K=64 accumulation pass of
    step 2) overlaps the k>=4 half of step 1 + its bounce write.
  step 2 (contract t): per c2 = (b, a1): two accumulating matmuls
    [K=64 (kappa half), M=128 (h',m)] x [K=64, 128 n] -> PSUM [64h'+m, n];
    bias fused into the PSUM->SBUF copy (DVE), output stored bf16 and
    upcast host-side.

All layout work (transposes, dense packing, final unshuffle) happens
host-side in numpy; the device only runs contiguous-ish DMAs, matmuls,
memsets and DVE copies.
"""

import numpy as np
import ml_dtypes

BS, TS, F, NF = 32, 64, 256, 128
NCORES = 8
B = BS // NCORES  # 4 local batches per core

F32 = np.float32
BF16 = ml_dtypes.bfloat16


# ---------------------------------------------------------------------------
# Host-side packing
# ---------------------------------------------------------------------------

def _pack_wt(conv_w: np.ndarray) -> np.ndarray:
    # WT[64q+y, 128p+n] = conv_w[n, 2p+q, y]
    wt = conv_w.transpose(1, 2, 0).reshape(32, 2, 64, NF)  # [p, q, y, n]
    wt = wt.transpose(1, 2, 0, 3)                          # [q, y, p, n]
    return np.ascontiguousarray(wt.reshape(128, 32 * NF), dtype=BF16)


def _pack_xk(xk_shard: np.ndarray) -> np.ndarray:
    # dense: XKD[64q+y, 16p+c] = xk[b, y, 8p+4q+a]   (c = 4b+a)
    xq = xk_shard.reshape(B, TS, 32, 2, 4)       # [b, y, p, q, a]
    src = xq.transpose(3, 1, 2, 0, 4)            # [q, y, p, b, a]
    return np.ascontiguousarray(src.reshape(128, 512), dtype=BF16)


def _pack_x0(x0_shard: np.ndarray) -> np.ndarray:
    # padded: X0L[kappa, (c2,h',m)] = x0[b, m, 4t+a] iff h'==h(kappa),
    # with c = 2c2+h = 4b+a, kappa = 64k2+32h+8k1k0+2j+q, t = 32k2+8k1k0+2j+q
    xt = x0_shard.reshape(B, TS, TS, 4).transpose(0, 3, 2, 1)  # [b, a, t, m]
    flat = xt.reshape(16, TS, TS)                              # [c, t, m]
    f = flat.reshape(8, 2, 2, 4, 4, 2, TS)       # [c2, h, k2, k10, j, q, m]
    x0d = f.transpose(2, 1, 3, 4, 5, 0, 6)       # [k2, h, k10, j, q, c2, m]
    x0l = np.zeros((2, 2, 32, 8, 2, TS), dtype=BF16)  # [k2,h,rest,c2,h',m]
    for h in range(2):
        x0l[:, h, :, :, h, :] = x0d.reshape(2, 2, 32, 8, TS)[:, h]
    return np.ascontiguousarray(x0l.reshape(128, 1024))


def _unpack_out(out_pack: np.ndarray, out_full: np.ndarray, r: int) -> None:
    # out_pack[64h+m, 128c2+n] = out[4r+b(c), n, a(c)*64+m], c = 2*c2+h
    o = np.asarray(out_pack, dtype=F32).reshape(2, TS, 8, NF)  # [h, m, c2, n]
    for c2 in range(8):
        for h in (0, 1):
            c = 2 * c2 + h
            b, a = divmod(c, 4)
            out_full[4 * r + b, :, a * TS:(a + 1) * TS] = o[h, :, c2, :].T


# ---------------------------------------------------------------------------
# Device program
# ---------------------------------------------------------------------------

_prog_cache = {}


def _emit_body_v11(nc, tc, pool, ps_pool, ps_pool2, in0_d, in1_d, out_d,
                   w2b_d):
    import concourse.mybir as mybir

    bf16 = mybir.dt.bfloat16
    f32 = mybir.dt.float32

    # PE warm-up: back-to-back matmuls on a zeroed bf16 tile while the
    # input DMAs stream in; gets the HAM clock gate to 2.4GHz before
    # step 1 starts.  No data deps -> scheduled first on PE.
    warm_s = pool.tile([128, 512], bf16, tag="warm")
    nc.gpsimd.memset(warm_s[:], 0.0)
    ps_w = ps_pool.tile([128, 512], f32, tag="warm_ps")
    for _ in range(6):
        nc.tensor.matmul(ps_w[:, :], warm_s[:, 0:128], warm_s[:, :],
                         start=True, stop=True)

    # ---- input DMAs split across both HWDGE rings so the two streams
    # run concurrently; chunks sized so both rings finish ~together ----
    wta = pool.tile([128, 1536], bf16, tag="wta")     # xk dense | wt pairs 0-7
    nc.sync.dma_start(wta[:], in0_d.ap()[:, 0:1536])
    wtb = pool.tile([128, 2048], bf16, tag="wtb")     # wt pairs 8-23
    nc.scalar.dma_start(wtb[:], in0_d.ap()[:, 1536:3584])
    wtc = pool.tile([128, 1024], bf16, tag="wtc")     # wt pairs 24-31
    nc.sync.dma_start(wtc[:], in0_d.ap()[:, 3584:4608])
    in1_s = pool.tile([128, 1152], bf16, tag="in1")   # x0 padded | bias
    nc.scalar.dma_start(in1_s[:], in1_d.ap())
    x0_pad = in1_s[:, 0:1024]

    # ---- zero-pad dense xk into its block-diagonal lhsT layout (DVE,
    # gates step-1; x0 ships pre-padded so step-2's stationary needs no
    # on-chip work that could contend with the psum casts) ----
    xk_pad = pool.tile([128, 1024], bf16, tag="xkpad")
    nc.gpsimd.memset(xk_pad[:], 0.0)
    for q in range(2):
        dst = xk_pad[64 * q:64 * (q + 1), :].rearrange(
            "p (a b) -> p a b", b=32)[:, :, 16 * q:16 * (q + 1)]
        src = wta[64 * q:64 * (q + 1), 0:512].rearrange(
            "p (a b) -> p a b", b=16)
        nc.vector.tensor_copy(dst, src)

    def wt_cols(p):  # rhs tile [128, 128] for pair p
        if p < 8:
            return wta[:, 512 + 128 * p:512 + 128 * (p + 1)]
        if p < 24:
            return wtb[:, 128 * (p - 8):128 * (p - 7)]
        return wtc[:, 128 * (p - 24):128 * (p - 23)]

    # ---- step 1: W2 = xk . wT, contract y (K = 128 = (q, y)) ----
    # psum tile u holds passes 4u..4u+3 at col 128*(k%4); each half is
    # cast-copied to bf16 and bounced out (contiguous 1KB runs) while
    # the other half computes.
    w2_s = pool.tile([128, 1024], bf16, tag="w2")
    w2r_s = pool.tile([128, 1024], bf16, tag="w2r")

    for u in range(2):
        ps1 = ps_pool.tile([128, 512], f32, tag="s1")
        for k in range(4 * u, 4 * u + 4):
            for j in range(4):
                p = 4 * k + j
                nc.tensor.matmul(
                    ps1[32 * j:32 * (j + 1), 128 * (k % 4):128 * (k % 4 + 1)],
                    xk_pad[:, 32 * p:32 * (p + 1)],
                    wt_cols(p),
                    start=True,
                    stop=True,
                    tile_position=(0, 32 * j),
                )
        nc.vector.tensor_copy(w2_s[:, 512 * u:512 * (u + 1)], ps1[:, :])
        # bounce out half u in two j-half writes on both rings at once:
        # src [p=(j,q,c2,h), (k,n)] scatters into the kappa-major dram
        # layout [h,k,j,q,c2,n] (k2 = u slice)
        for v in range(2):
            srcA = w2_s[64 * v:64 * (v + 1),
                        512 * u:512 * (u + 1)].rearrange(
                "p (k n) -> p k n", k=4)
            dstA = w2b_d.ap()[u, :, :, 2 * v:2 * (v + 1)].rearrange(
                "h k j q c2 n -> j q c2 h k n")
            (nc.sync if v == 0 else nc.scalar).dma_start(dstA, srcA)
    for u in range(2):
        # readback half u: kappa-major layout makes this a plain
        # contiguous [64, 1024] load into partitions [64u, 64u+64);
        # the two halves stream on both rings concurrently
        (nc.sync if u == 0 else nc.scalar).dma_start(
            w2r_s[64 * u:64 * (u + 1), :], w2b_d.ap()[u])


    # ---- step 2: out = x0 . W2, contract t (K = 128 = kappa) ----
    # quarter-bank psum tiles so each bias-add + out DMA fires after
    # just two matmuls
    out_s = pool.tile([128, 1024], bf16, tag="out")
    # bf16 bias view fed straight into the adds - no on-chip cast op
    bias2 = in1_s[:, 1024:1152].unsqueeze(1).broadcast_to([128, 2, 128])
    for v in range(4):
        ps2 = ps_pool2.tile([128, 256], f32, tag="s2")
        for c2 in range(2 * v, 2 * v + 2):
            nc.tensor.matmul(
                ps2[:, 128 * (c2 % 2):128 * (c2 % 2 + 1)],
                x0_pad[:, 128 * c2:128 * (c2 + 1)],
                w2r_s[:, 128 * c2:128 * (c2 + 1)],
                start=True,
                stop=True,
            )
        nc.vector.tensor_add(
            out_s[:, 256 * v:256 * (v + 1)].rearrange("p (f n) -> p f n", f=2),
            ps2[:, :].rearrange("p (f n) -> p f n", f=2),
            bias2,
        )
        sl = slice(256 * v, 256 * (v + 1))
        (nc.sync if v % 2 == 0 else nc.scalar).dma_start(
            out_d.ap()[:, sl], out_s[:, sl])


def _build_program(version=21):
    if version in _prog_cache:
        return _prog_cache[version]

    from contextlib import ExitStack

    import concourse.bacc as bacc
    import concourse.mybir as mybir
    import concourse.tile as tile

    bf16 = mybir.dt.bfloat16
    nc = bacc.Bacc("TRN2", target_bir_lowering=False, debug=False)

    # in0 = [xk_dense (512) | wt (4096)], in1 = [x0_dense (512) | bias (128)]
    in0_d = nc.dram_tensor("in0_pack", [128, 4608], bf16, kind="ExternalInput")
    in1_d = nc.dram_tensor("in1_pack", [128, 1152], bf16, kind="ExternalInput")
    out_d = nc.dram_tensor("out_pack", [128, 1024], bf16, kind="ExternalOutput")
    # bounce layout [k2, h, k10, j, q, c2, n] (kappa-major)
    w2b_d = nc.dram_tensor("w2_bounce", [2, 2, 4, 4, 2, 8, 128], bf16)

    with tile.TileContext(nc) as tc, ExitStack() as ctx:
        pool = ctx.enter_context(tc.tile_pool(name="io", bufs=1))
        ps_pool = ctx.enter_context(tc.tile_pool(name="ps", bufs=2, space="PSUM"))
        ps_pool2 = ctx.enter_context(tc.tile_pool(name="ps2", bufs=4, space="PSUM"))
        _emit_body_v11(nc, tc, pool, ps_pool, ps_pool2, in0_d, in1_d, out_d,
                       w2b_d)

    nc.compile()
    _prog_cache[version] = nc
    return nc


def pack_core_inputs(x_0, x_k, conv_w, conv_b, version=21):
    """Returns (in_maps list of 8 dicts) for run_bass_kernel_spmd."""
    wt = _pack_wt(np.asarray(conv_w, dtype=F32))
    bias = np.ascontiguousarray(
        np.broadcast_to(
            np.asarray(conv_b, dtype=F32).astype(BF16), (128, 128)))
    x0 = np.asarray(x_0, dtype=F32)
    xk = np.asarray(x_k, dtype=F32)
    in_maps = []
    for r in range(NCORES):
        in0 = np.concatenate([_pack_xk(xk[B * r:B * (r + 1)]), wt], axis=1)
        in1 = np.concatenate([_pack_x0(x0[B * r:B * (r + 1)]), bias], axis=1)
        in_maps.append({
            "in0_pack": np.ascontiguousarray(in0),
            "in1_pack": np.ascontiguousarray(in1),
        })
    return in_maps


VERSION = 21  # current best variant


def kernel(x_0, x_k, conv_w, conv_b):
    from concourse.bass_utils import run_bass_kernel_spmd

    nc = _build_program(VERSION)
    in_maps = pack_core_inputs(x_0, x_k, conv_w, conv_b, version=VERSION)
    res = run_bass_kernel_spmd(nc, in_maps, core_ids=list(range(NCORES)))
    out = np.empty((BS, NF, F), dtype=F32)
    for r in range(NCORES):
        _unpack_out(res.results[r]["out_pack"], out, r)
    return out


# ---------------------------------------------------------------------------
# numpy model of the packed device program (for testing the packing logic)
# ---------------------------------------------------------------------------

def _numpy_model(x_0, x_k, conv_w, conv_b):
    out = np.empty((BS, NF, F), dtype=F32)
    in_maps = pack_core_inputs(x_0, x_k, conv_w, conv_b)
    for r in range(NCORES):
        m = in_maps[r]
        xkd = np.asarray(m["in0_pack"][:, :512], dtype=F32)
        wt = np.asarray(m["in0_pack"][:, 512:], dtype=F32)
        x0_pad = np.asarray(m["in1_pack"][:, :1024], dtype=F32)
        bias = np.asarray(m["in1_pack"][:, 1024:1152], dtype=F32)
        # on-chip xk pad
        xk_pad = np.zeros((128, 1024), dtype=F32)
        for q in range(2):
            s = xkd[64 * q:64 * (q + 1)].reshape(64, 32, 16)
            d = xk_pad[64 * q:64 * (q + 1)].reshape(64, 32, 32)
            d[:, :, 16 * q:16 * (q + 1)] = s
        # step 1
        w2 = np.zeros((128, 1024), dtype=F32)
        for k in range(8):
            for j in range(4):
                p = 4 * k + j
                w2[32 * j:32 * (j + 1), 128 * k:128 * (k + 1)] = (
                    xk_pad[:, 32 * p:32 * (p + 1)].T
                    @ wt[:, 128 * p:128 * (p + 1)])
        # bounce: src partition (j,q,c2,h), free (k,n) -> [j,q,c2,h,k,n]
        w2b = w2.reshape(4, 2, 8, 2, 8, 128)
        # readback: dst partition (h,k,j,q) with k = 4*k2 + k10 and the
        # k2 bit on top: kappa = (k2, h, k10, j, q)
        w2r = np.empty((2, 2, 4, 4, 2, 8, 128), dtype=F32)  # [k2,h,k10,j,q,c2,n]
        for k2 in range(2):
            # srcB slice [j,q,c2,h,k10,n] -> (h k10 j q) c2 n
            sl = w2b[:, :, :, :, 4 * k2:4 * (k2 + 1), :]
            w2r[k2] = sl.transpose(3, 4, 0, 1, 2, 5)
        w2r = w2r.reshape(128, 1024)
        # step 2
        out_pack = np.zeros((128, 1024), dtype=F32)
        for c2 in range(8):
            out_pack[:, 128 * c2:128 * (c2 + 1)] = (
                x0_pad[:, 128 * c2:128 * (c2 + 1)].T
                @ w2r[:, 128 * c2:128 * (c2 + 1)] + bias)
        _unpack_out(out_pack, out, r)
    return out
